# revision 1
# baseline (speedup 1.0000x reference)
"""Trainium2 Bass kernel for BronxModel (GNN message passing SDE).

Strategy (8 NeuronCores, SPMD):
  - Nodes dst-sharded across cores; within a core, dst nodes sorted by
    in-degree, tiled 128/partition. One extra all-zero tile per core is the
    gather pad target (its messages stay zero).
  - State y in SBUF. Per SDE step: per-tile PE matmul y@W_msg scaled by
    rsqrt(deg_out) -> bf16 message table shard; AllGather the table.
  - Hybrid gather of per-edge message rows (the bandwidth-critical part):
      * per-column indirect DMA (128 rows/call, Pool-engine bound) for the
        first KCAP slots of every (tile, partition);
      * dma_gather pair-fetch (DMA-engine bound) for remaining edges: the
        bf16 table viewed as [pairs, 128] 256B rows, int16 bank-local pair
        indices split by (bank, parity); the strided DVE reduce reads only
        the correct 64-element parity half. Calls are capped by the SWDGE
        descriptor-ring size (dynamic_dma_scratch_size).
    The two paths saturate different engines concurrently.
  - Segment sum via strided tensor_reduce; fused y update.
  - h @ W_in on host; final y @ W_out on device.
"""
import os
import sys

sys.path.insert(0, "/opt/trn_rl_repo")

import numpy as np
import ml_dtypes

import concourse.bass as bass
import concourse.bacc as bacc
import concourse.tile as tile
import concourse.mybir as mybir
from concourse import bass_utils

NCORES = 8
P = 128
KCAP = 10         # slots per (tile,partition) routed to the per-column path
NBANKS = 2        # int16 pair banks (signed-safe: pairs/bank <= 32767)
CCMAX = 8         # max pair columns per dma_gather call (1024 descs = HW ring cap)
SUBNT = 2         # tiles per pair-reduce subchunk
NTC = 8           # tiles per percol chunk
NQ = 3            # SWDGE queues: percol on 0, pair calls alternate 1/2


def _preprocess(h, W_in, W_msg, W_out, dW, src, dst):
    N = h.shape[0]
    E = src.shape[0]
    HID = W_msg.shape[0]
    STEPS = dW.shape[0]
    OUTF = W_out.shape[1]
    DT = 1.0 / STEPS
    SIGMA = 0.01
    sqrt_dt = np.sqrt(DT).astype(np.float32)

    nsh = (N + NCORES - 1) // NCORES
    T = (nsh + P - 1) // P
    T2 = T + 1
    npad = T2 * P
    W = T2 * HID
    NPAIR_ALL = NCORES * npad // 2
    BANKP = (NPAIR_ALL + NBANKS - 1) // NBANKS
    NG = NBANKS * 2

    ones = np.ones(E, np.float32)
    deg_out = np.zeros(N, np.float32)
    np.add.at(deg_out, src, ones)
    deg_in = np.zeros(N, np.float32)
    np.add.at(deg_in, dst, ones)
    rdo = 1.0 / np.sqrt(np.maximum(deg_out, 1.0))
    rdi = 1.0 / np.sqrt(np.maximum(deg_in, 1.0))

    x0 = (h.astype(np.float32) @ W_in.astype(np.float32)).astype(np.float32)

    ords = []
    pos_of = np.zeros(N, np.int64)
    for c in range(NCORES):
        lo, hi = c * nsh, min((c + 1) * nsh, N)
        nodes = np.arange(lo, hi)
        o = nodes[np.argsort(deg_in[nodes], kind="stable")]
        ords.append(o)
        pos_of[o] = np.arange(len(o))
    n_owner = np.minimum(np.arange(N) // nsh, NCORES - 1)
    table_row = n_owner * npad + (pos_of % P) * T2 + pos_of // P
    TBL = NCORES * npad
    ZROW = T

    pair_of = table_row // 2
    par_of = table_row % 2
    grp_of = np.minimum(pair_of // BANKP, NBANKS - 1) * 2 + par_of

    # zero-pad pair per (bank, parity)
    pad_pair = np.zeros((NBANKS, 2), np.int64)
    for b in range(NBANKS):
        for q in range(2):
            found = False
            for c in range(NCORES):
                for p in range(P):
                    r = c * npad + p * T2 + T
                    if min((r // 2) // BANKP, NBANKS - 1) == b and r % 2 == q:
                        pad_pair[b, q] = r // 2 - b * BANKP
                        found = True
                        break
                if found:
                    break
            assert found

    # per-core edge arrays
    percore = []
    for c in range(NCORES):
        m = np.minimum(dst // nsh, NCORES - 1) == c
        s_c = src[m]
        pos = pos_of[dst[m]]
        order = np.argsort(pos, kind="stable")
        s_c = s_c[order]
        pos = pos[order]
        percore.append((s_c, (pos // P).astype(np.int64), (pos % P).astype(np.int64),
                        grp_of[s_c]))

    # shared tail-L bounds per (tile, group) after KCAP water-filling
    Ltail = np.zeros((T, NG), np.int64)
    for c in range(NCORES):
        s_c, t_arr, p_arr, g_arr = percore[c]
        cnt4 = np.zeros((T, P, NG), np.int64)
        np.add.at(cnt4, (t_arr, p_arr, g_arr), 1)
        c4 = cnt4.copy()
        for _ in range(KCAP):
            gmax = c4.argmax(axis=2)
            mx = np.take_along_axis(c4, gmax[:, :, None], axis=2)
            dec = (mx > 0).astype(np.int64)
            np.put_along_axis(c4, gmax[:, :, None], mx - dec, axis=2)
        Ltail = np.maximum(Ltail, c4.max(axis=1))

    # subchunks (SUBNT tiles) with per-group uniform Lg
    NSUB = (T + SUBNT - 1) // SUBNT
    subLg = np.zeros((NSUB, NG), np.int64)
    for s in range(NSUB):
        t0 = s * SUBNT
        subLg[s] = Ltail[t0 : t0 + SUBNT].max(axis=0)

    # pair-gather calls: (sub, g, l0, Lr); nominal range width LRN per sub
    calls = []          # dicts
    pidx_base = 0
    call_lut = {}       # (sub, g, k) -> call index
    for s in range(NSUB):
        t0 = s * SUBNT
        nts = min(SUBNT, T - t0)
        lrn = max(1, CCMAX // nts)
        for g in range(NG):
            Lg = int(subLg[s, g])
            k = 0
            l0 = 0
            while l0 < Lg:
                Lr = min(lrn, Lg - l0)
                calls.append(dict(s=s, t0=t0, nts=nts, g=g, l0=l0, Lr=Lr,
                                  base=pidx_base))
                call_lut[(s, g, k)] = len(calls) - 1
                pidx_base += nts * Lr * P
                l0 += Lr
                k += 1
    TOTIDX = pidx_base

    # percol chunks
    pchunks = []
    for t0 in range(0, T, NTC):
        nt = min(NTC, T - t0)
        pchunks.append(dict(t0=t0, nt=nt, pc_base=t0 * KCAP))
    PCT = T * KCAP

    # order of work per percol chunk: the pair calls whose sub lies inside.
    # calls are sub-major, so each chunk's pidx range is contiguous.
    for pc in pchunks:
        lo, hi = pc["t0"], pc["t0"] + pc["nt"]
        pc["calls"] = [i for i, cl in enumerate(calls)
                       if lo <= cl["t0"] and cl["t0"] < hi]
        if pc["calls"]:
            pc["pix_lo"] = calls[pc["calls"][0]]["base"]
            last = calls[pc["calls"][-1]]
            pc["pix_hi"] = last["base"] + last["nts"] * last["Lr"] * P
        else:
            pc["pix_lo"] = pc["pix_hi"] = 0

    in_maps = []
    dWs = (dW.astype(np.float32) * (SIGMA * sqrt_dt)).astype(np.float32)

    # per-edge lookup tables
    lut_callid = np.full((NSUB, NG, 16), -1, np.int64)
    lut_base = np.zeros((NSUB, NG, 16), np.int64)
    lut_l0 = np.zeros((NSUB, NG, 16), np.int64)
    lut_Lr = np.ones((NSUB, NG, 16), np.int64)
    for i, cl in enumerate(calls):
        s, g = cl["s"], cl["g"]
        t0 = s * SUBNT
        nts = cl["nts"]
        lrn = max(1, CCMAX // nts)
        k = cl["l0"] // lrn
        lut_callid[s, g, k] = i
        lut_base[s, g, k] = cl["base"]
        lut_l0[s, g, k] = cl["l0"]
        lut_Lr[s, g, k] = cl["Lr"]

    for c in range(NCORES):
        s_c, t_arr, p_arr, g_arr = percore[c]
        trows = table_row[s_c]
        cnt4 = np.zeros((T, P, NG), np.int64)
        np.add.at(cnt4, (t_arr, p_arr, g_arr), 1)
        take = np.zeros((T, P, NG), np.int64)
        c4 = cnt4.copy()
        for _ in range(KCAP):
            gmax = c4.argmax(axis=2)
            mx = np.take_along_axis(c4, gmax[:, :, None], axis=2)
            dec = (mx > 0).astype(np.int64)
            np.put_along_axis(c4, gmax[:, :, None], mx - dec, axis=2)
            tk = np.take_along_axis(take, gmax[:, :, None], axis=2)
            np.put_along_axis(take, gmax[:, :, None], tk + dec, axis=2)

        key = (t_arr * P + p_arr) * NG + g_arr
        korder = np.argsort(key, kind="stable")
        kk = key[korder]
        first = np.concatenate([[True], kk[1:] != kk[:-1]])
        idxs = np.arange(len(kk))
        start = np.maximum.accumulate(np.where(first, idxs, 0))
        rank = np.zeros(len(kk), np.int64)
        rank[korder] = idxs - start

        is_pc = rank < take[t_arr, p_arr, g_arr]
        key2 = t_arr * P + p_arr
        pcslot = np.full(len(s_c), -1, np.int64)
        sel = np.where(is_pc)[0]
        o2 = sel[np.argsort(key2[sel], kind="stable")]
        k2 = key2[o2]
        f2 = np.concatenate([[True], k2[1:] != k2[:-1]])
        i2 = np.arange(len(o2))
        st2 = np.maximum.accumulate(np.where(f2, i2, 0))
        pcslot[o2] = i2 - st2
        tlslot = np.full(len(s_c), -1, np.int64)
        nsel = np.where(~is_pc)[0]
        o3 = nsel[np.argsort(key[nsel], kind="stable")]
        k3 = key[o3]
        f3 = np.concatenate([[True], k3[1:] != k3[:-1]])
        i3 = np.arange(len(o3))
        st3 = np.maximum.accumulate(np.where(f3, i3, 0))
        tlslot[o3] = i3 - st3

        grid = np.full((P, PCT), ZROW, np.int32)
        cols = t_arr * KCAP + pcslot
        gm = is_pc
        grid[p_arr[gm], cols[gm]] = trows[gm].astype(np.int32)

        pidx = np.zeros((P, TOTIDX // 16), np.int16)
        # pads
        for cl in calls:
            b, q = divmod(cl["g"], 2)
            n = cl["nts"] * cl["Lr"] * P
            sl = np.arange(cl["base"], cl["base"] + n)
            val = np.int16(pad_pair[b, q])
            for rep in range(8):
                pidx[(sl % 16) + rep * 16, sl // 16] = val
        # real edges
        tm = ~is_pc
        et, ep, eg, es = t_arr[tm], p_arr[tm], g_arr[tm], tlslot[tm]
        epair = pair_of[s_c[tm]]
        esub = et // SUBNT
        nts_v = np.minimum(SUBNT, T - esub * SUBNT)
        lrn_v = np.maximum(1, CCMAX // nts_v)
        kv = es // lrn_v
        basev = lut_base[esub, eg, kv]
        l0v = lut_l0[esub, eg, kv]
        Lrv = lut_Lr[esub, eg, kv]
        col = (et - esub * SUBNT) * Lrv + (es - l0v)
        slot = basev + col * P + ep
        bankv = eg // 2
        v16 = (epair - bankv * BANKP).astype(np.int16)
        for rep in range(8):
            pidx[(slot % 16) + rep * 16, slot // 16] = v16

        o = ords[c]
        nreal = len(o)
        ytmp = np.zeros((npad, HID), np.float32)
        ytmp[:nreal] = x0[o]
        y0 = ytmp.reshape(T2, P, HID).transpose(1, 0, 2).reshape(P, W)
        rv = np.zeros(npad, np.float32)
        rv[:nreal] = rdi[o] * DT
        rdi_full = np.repeat(rv, HID).reshape(T2, P, HID).transpose(1, 0, 2).reshape(P, W)
        rdi_full = rdi_full.astype(ml_dtypes.bfloat16)
        ro = np.zeros(T * P, np.float32)
        ro[:nreal] = rdo[o]
        rdo_col = ro.reshape(T, P).T.copy()
        dtmp = np.zeros((STEPS, npad, HID), np.float32)
        dtmp[:, :nreal] = dWs[:, o, :]
        dwc = (
            dtmp.reshape(STEPS, T2, P, HID)
            .transpose(0, 2, 1, 3)
            .reshape(STEPS, P, W)
            .astype(ml_dtypes.bfloat16)
        )
        in_maps.append(
            {
                "y0_in": np.ascontiguousarray(y0),
                "dw_in": np.ascontiguousarray(dwc),
                "rdi_in": np.ascontiguousarray(rdi_full),
                "rdo_in": np.ascontiguousarray(rdo_col),
                "grid_in": grid,
                "pidx_in": pidx,
                "wmsg_in": W_msg.astype(np.float32),
                "wout_in": W_out.astype(np.float32),
            }
        )

    meta = dict(
        N=N, HID=HID, OUTF=OUTF, STEPS=STEPS, DT=DT, T=T, T2=T2, npad=npad,
        TBL=TBL, PCT=PCT, TOTIDX=TOTIDX, BANKP=BANKP, calls=calls,
        pchunks=pchunks, ords=ords, nsh=nsh,
    )
    return in_maps, meta


def _build(meta):
    from concourse.masks import make_identity

    HID, OUTF, STEPS, DT = meta["HID"], meta["OUTF"], meta["STEPS"], meta["DT"]
    T, T2, TBL = meta["T"], meta["T2"], meta["TBL"]
    PCT, TOTIDX, BANKP = meta["PCT"], meta["TOTIDX"], meta["BANKP"]
    calls, pchunks = meta["calls"], meta["pchunks"]
    W = T2 * HID
    QT = (T + 3) // 4           # output staged in quarters
    HOUT = QT * OUTF
    PIXMAX = max((pc["pix_hi"] - pc["pix_lo"]) // 16 for pc in pchunks)
    MAXCALLS = max(len(pc["calls"]) for pc in pchunks)

    nc = bacc.Bacc("TRN2", target_bir_lowering=False, debug=False,
                   num_devices=NCORES, num_swdge_queues=NQ)
    y0_in = nc.dram_tensor("y0_in", [P, W], mybir.dt.float32, kind="ExternalInput")
    dw_in = nc.dram_tensor("dw_in", [STEPS, P, W], mybir.dt.bfloat16, kind="ExternalInput")
    rdi_in = nc.dram_tensor("rdi_in", [P, W], mybir.dt.bfloat16, kind="ExternalInput")
    rdo_in = nc.dram_tensor("rdo_in", [P, T], mybir.dt.float32, kind="ExternalInput")
    grid_in = nc.dram_tensor("grid_in", [P, PCT], mybir.dt.int32, kind="ExternalInput")
    pidx_in = nc.dram_tensor("pidx_in", [P, TOTIDX // 16], mybir.dt.int16, kind="ExternalInput")
    wmsg_in = nc.dram_tensor("wmsg_in", [HID, HID], mybir.dt.float32, kind="ExternalInput")
    wout_in = nc.dram_tensor("wout_in", [HID, OUTF], mybir.dt.float32, kind="ExternalInput")
    out_d = nc.dram_tensor("out_d", [P, T * OUTF], mybir.dt.float32, kind="ExternalOutput")

    with tile.TileContext(nc) as tc:
        with (
            tc.tile_pool(name="state", bufs=1) as st,
            tc.tile_pool(name="work", bufs=3) as wk,
            tc.tile_pool(name="dwp", bufs=2) as dwp,
            tc.tile_pool(name="pcp", bufs=2) as pcp,
            tc.tile_pool(name="prp", bufs=6) as prp,
            tc.tile_pool(name="pxp", bufs=3) as pxp,
            tc.tile_pool(name="ptp", bufs=MAXCALLS + 4) as ptp,
            tc.tile_pool(name="osp", bufs=1) as osp,
            tc.tile_pool(name="psum", bufs=2, space="PSUM") as ps,
            tc.tile_pool(name="dram", bufs=1, space="DRAM") as dram,
        ):
            y = st.tile([P, W], mybir.dt.float32)
            nc.sync.dma_start(y[:], y0_in[:, :])
            rdi_t = st.tile([P, W], mybir.dt.bfloat16)
            nc.sync.dma_start(rdi_t[:], rdi_in[:, :])
            rdo_t = st.tile([P, T], mybir.dt.float32)
            nc.sync.dma_start(rdo_t[:], rdo_in[:, :])
            grid_t = st.tile([P, PCT], mybir.dt.int32)
            nc.sync.dma_start(grid_t[:], grid_in[:, :])
            wmsg = st.tile([HID, HID], mybir.dt.float32)
            nc.sync.dma_start(wmsg[:], wmsg_in[:, :])
            wout = st.tile([HID, OUTF], mybir.dt.float32)
            nc.sync.dma_start(wout[:], wout_in[:, :])
            ident = st.tile([P, P], mybir.dt.float32)
            make_identity(nc, ident[:])
            m_stage = st.tile([P, W], mybir.dt.bfloat16)
            nc.vector.memset(m_stage[:], 0.0)
            agg = st.tile([P, W], mybir.dt.float32)
            nc.vector.memset(agg[:], 0.0)
            agg2 = st.tile([P, W], mybir.dt.float32)
            nc.vector.memset(agg2[:], 0.0)

            table = dram.tile([TBL, HID], mybir.dt.bfloat16)
            bounce = dram.tile([P, W], mybir.dt.bfloat16)
            tpair = table[:, :].rearrange("(r two) h -> r (two h)", two=2)

            def emit_msgs(t0, nt):
                """Stage messages m = (y @ Wmsg)*rdo for tiles [t0, t0+nt)."""
                for t in range(t0, t0 + nt):
                    ytp = ps.tile([HID, P], mybir.dt.float32, space="PSUM", tag="ytp")
                    nc.tensor.transpose(
                        out=ytp[:], in_=y[:, t * HID : (t + 1) * HID], identity=ident[:]
                    )
                    yT = wk.tile([HID, P], mybir.dt.float32, tag="yT")
                    nc.scalar.activation(yT[:], ytp[:], mybir.ActivationFunctionType.Copy)
                    mp = ps.tile([P, HID], mybir.dt.float32, space="PSUM", tag="mp")
                    nc.tensor.matmul(out=mp[:], lhsT=yT[:], rhs=wmsg[:], start=True, stop=True)
                    nc.scalar.activation(
                        m_stage[:, t * HID : (t + 1) * HID],
                        mp[:],
                        mybir.ActivationFunctionType.Copy,
                        scale=rdo_t[:, t : t + 1],
                    )

            def emit_ag():
                nc.sync.dma_start(bounce[:], m_stage[:])
                nc.gpsimd.collective_compute(
                    "AllGather",
                    mybir.AluOpType.bypass,
                    replica_groups=[list(range(NCORES))],
                    ins=[bounce[:]],
                    outs=[table[0:TBL, :]],
                )

            # prologue: messages for step 0
            emit_msgs(0, T)
            emit_ag()

            for k in range(STEPS):
                dwk = dwp.tile([P, W], mybir.dt.bfloat16, tag="dw")
                nc.sync.dma_start(dwk[:], dw_in[k, :, :])
                nc.vector.memset(agg2[:, 0 : T * HID], 0.0)
                for pc in pchunks:
                    t0, nt = pc["t0"], pc["nt"]
                    npc = nt * KCAP
                    pcb = pc["pc_base"]
                    pix_lo, pix_hi = pc["pix_lo"], pc["pix_hi"]
                    span = (pix_hi - pix_lo) // 16
                    pix = pxp.tile([P, PIXMAX], mybir.dt.int16, tag="pix")
                    if span > 0:
                        nc.sync.dma_start(
                            pix[:, 0:span], pidx_in[:, pix_lo // 16 : pix_hi // 16]
                        )
                    mpc = pcp.tile([P, NTC * KCAP * HID], mybir.dt.bfloat16, tag="mpc")
                    subw = SUBNT * KCAP
                    for j in range(npc):
                        nc.gpsimd.indirect_dma_start(
                            out=mpc[:, j * HID : (j + 1) * HID],
                            out_offset=None,
                            in_=table[:, :],
                            in_offset=bass.IndirectOffsetOnAxis(
                                ap=grid_t[:, pcb + j : pcb + j + 1], axis=0
                            ),
                        )
                        # percol reduce per SUBNT tiles at sub boundary so the
                        # DVE frees mpc promptly
                        if (j + 1) % subw == 0 or j == npc - 1:
                            sub0 = (j // subw) * subw
                            nts2 = (j + 1 - sub0) // KCAP
                            ts0 = t0 + sub0 // KCAP
                            nc.vector.tensor_reduce(
                                out=agg[:, ts0 * HID : (ts0 + nts2) * HID].rearrange(
                                    "p (t h) -> p t h", t=nts2
                                ),
                                in_=mpc[:, sub0 * HID : (j + 1) * HID].rearrange(
                                    "p (t l h) -> p t h l", t=nts2, l=KCAP
                                ),
                                axis=mybir.AxisListType.X,
                                op=mybir.AluOpType.add,
                            )
                    for ii, ci in enumerate(pc["calls"]):
                        cl = calls[ci]
                        nts, Lr, g = cl["nts"], cl["Lr"], cl["g"]
                        b, q = divmod(g, 2)
                        ncols = nts * Lr
                        nidx = ncols * P
                        ib = cl["base"] - pix_lo
                        pout = prp.tile([P, CCMAX * 2 * HID], mybir.dt.bfloat16, tag="pout")
                        nc.gpsimd.dma_gather(
                            out_ap=pout[:, 0 : ncols * 2 * HID].rearrange(
                                "p (c e) -> p c e", c=ncols
                            ),
                            in_ap=tpair[b * BANKP : (b + 1) * BANKP, :],
                            idxs_ap=pix[:, ib // 16 : (ib + nidx) // 16],
                            num_idxs=nidx,
                            num_idxs_reg=nidx,
                            elem_size=2 * HID,
                            queue_num=1 + (ci % (NQ - 1)),
                        )
                        ptmp = ptp.tile([P, SUBNT * HID], mybir.dt.float32, tag="ptmp")
                        nc.vector.tensor_reduce(
                            out=ptmp[:, 0 : nts * HID].rearrange("p (t h) -> p t h", t=nts),
                            in_=pout[:, 0 : ncols * 2 * HID]
                            .rearrange("p (t l e) -> p t l e", t=nts, l=Lr)[
                                :, :, :, q * HID : (q + 1) * HID
                            ]
                            .rearrange("p t l h -> p t h l"),
                            axis=mybir.AxisListType.X,
                            op=mybir.AluOpType.add,
                        )
                        asl = agg2[:, cl["t0"] * HID : (cl["t0"] + nts) * HID]
                        nc.vector.tensor_add(asl, asl, ptmp[:, 0 : nts * HID])
                    # per-chunk update + next-step messages (overlaps later
                    # chunks' gathers on DVE/PE/Act while Pool keeps going)
                    sl = slice(t0 * HID, (t0 + nt) * HID)
                    nc.vector.tensor_add(agg[:, sl], agg[:, sl], agg2[:, sl])
                    nc.vector.tensor_mul(agg[:, sl], agg[:, sl], rdi_t[:, sl])
                    nc.vector.scalar_tensor_tensor(
                        out=y[:, sl], in0=y[:, sl], scalar=1.0 - DT,
                        in1=agg[:, sl],
                        op0=mybir.AluOpType.mult, op1=mybir.AluOpType.add,
                    )
                    nc.vector.tensor_add(y[:, sl], y[:, sl], dwk[:, sl])
                    if k < STEPS - 1:
                        emit_msgs(t0, nt)
                if k < STEPS - 1:
                    emit_ag()

            for hstart in range(0, T, QT):
                hend = min(hstart + QT, T)
                ostage = osp.tile([P, HOUT], mybir.dt.float32, tag="ostage")
                for t in range(hstart, hend):
                    ytp = ps.tile([HID, P], mybir.dt.float32, space="PSUM", tag="ytp")
                    nc.tensor.transpose(
                        out=ytp[:], in_=y[:, t * HID : (t + 1) * HID], identity=ident[:]
                    )
                    yT = wk.tile([HID, P], mybir.dt.float32, tag="yT")
                    nc.scalar.activation(yT[:], ytp[:], mybir.ActivationFunctionType.Copy)
                    op = ps.tile([P, OUTF], mybir.dt.float32, space="PSUM", tag="op")
                    nc.tensor.matmul(out=op[:], lhsT=yT[:], rhs=wout[:], start=True, stop=True)
                    nc.scalar.activation(
                        ostage[:, (t - hstart) * OUTF : (t - hstart + 1) * OUTF],
                        op[:],
                        mybir.ActivationFunctionType.Copy,
                    )
                nc.sync.dma_start(
                    out_d[:, hstart * OUTF : hend * OUTF],
                    ostage[:, 0 : (hend - hstart) * OUTF],
                )

    nc.compile()
    return nc


def kernel(h, W_in, W_msg, W_out, dW, src, dst):
    h = np.asarray(h)
    W_in = np.asarray(W_in)
    W_msg = np.asarray(W_msg)
    W_out = np.asarray(W_out)
    dW = np.asarray(dW)
    src = np.asarray(src)
    dst = np.asarray(dst)

    in_maps, meta = _preprocess(h, W_in, W_msg, W_out, dW, src, dst)
    nc = _build(meta)

    trace = bool(int(os.environ.get("BASS_KERNEL_TRACE", "0")))
    res = bass_utils.run_bass_kernel_spmd(
        nc, in_maps, core_ids=list(range(NCORES)), trace=trace
    )
    if trace and res.exec_time_ns:
        print(f"HW exec time: {res.exec_time_ns} ns")

    N, OUTF, T = meta["N"], meta["OUTF"], meta["T"]
    out = np.zeros((N, OUTF), np.float32)
    for c in range(NCORES):
        o = meta["ords"][c]
        dev = res.results[c]["out_d"]
        dev = dev.reshape(P, T, OUTF).transpose(1, 0, 2).reshape(T * P, OUTF)
        out[o] = dev[: len(o)]
    return out



# revision 4
# speedup vs baseline: 1.4179x; 1.4179x over previous
"""Trainium2 Bass kernel for BronxModel (GNN message passing SDE).

Strategy (8 NeuronCores, SPMD, dst-sharded nodes):
  - Per SDE step: per-tile PE matmul y@W_msg scaled by rsqrt(deg_out) ->
    bf16 message table shard; AllGather -> DRAM table, viewed as 256B
    pair rows [NPAIR, 128].
  - All per-edge fetches go through dma_gather (packed SWDGE descriptors,
    1024 idxs/call — the HW cap): idx i -> out (partition i%128, column
    i//128) of a chunk staging tile. int16 idxs force 2 banks x row
    parity = 4 edge groups; a slot grid per group with per-tile uniform
    depth L_g(t). Nodes are re-ordered within (bank,parity) row-classes
    by their group-count vectors so tiles hold vector-similar nodes,
    minimizing the max-over-partitions padding.
  - Segment sum: per (chunk, group), an in-place contiguous bf16
    tensor_tensor tree over k-major full 256B elements; the row-parity
    half is selected only at the final fp32 combine. Fused y update.
  - h @ W_in on host; final y @ W_out on device.
"""
import os
import sys

sys.path.insert(0, "/opt/trn_rl_repo")

import numpy as np
import ml_dtypes

import concourse.bass as bass
import concourse.bacc as bacc
import concourse.tile as tile
import concourse.mybir as mybir
from concourse import bass_utils

NCORES = 8
P = 128
NBANKS = 2
CALL_COLS = 8       # grid columns per dma_gather call (8*128 = 1024 idx cap)
MAXNT = 6           # tiles per chunk
MPC_COLS = 152      # max grid columns per chunk (mpc: cols*256B/partition)
NQ = 4              # SWDGE queues; gathers rotate on 1..3


def _preprocess(h, W_in, W_msg, W_out, dW, src, dst):
    N = h.shape[0]
    HID = W_msg.shape[0]
    STEPS = dW.shape[0]
    DT = 1.0 / STEPS
    SIGMA = 0.01
    sqrt_dt = np.sqrt(DT).astype(np.float32)

    nsh = (N + NCORES - 1) // NCORES
    T = (nsh + P - 1) // P
    T2 = T + 1
    npad = T2 * P
    TBL = NCORES * npad
    NPAIR = TBL // 2
    BANKP = (NPAIR + NBANKS - 1) // NBANKS
    WY = T * HID
    W2 = T2 * HID

    ones = np.ones(src.shape[0], np.float32)
    deg_out = np.zeros(N, np.float32)
    np.add.at(deg_out, src, ones)
    deg_in = np.zeros(N, np.float32)
    np.add.at(deg_in, dst, ones)
    rdo = 1.0 / np.sqrt(np.maximum(deg_out, 1.0))
    rdi = 1.0 / np.sqrt(np.maximum(deg_in, 1.0))

    x0 = h.astype(np.float32) @ W_in.astype(np.float32)

    n_owner = np.minimum(np.arange(N) // nsh, NCORES - 1)

    # ---- phase 1: provisional order by in-degree -> frozen group labels
    pos1 = np.zeros(N, np.int64)
    for c in range(NCORES):
        lo, hi = c * nsh, min((c + 1) * nsh, N)
        nodes = np.arange(lo, hi)
        o = nodes[np.argsort(deg_in[nodes], kind="stable")]
        pos1[o] = np.arange(len(o))
    row1 = n_owner * npad + (pos1 % P) * T2 + pos1 // P
    bank1 = np.minimum((row1 // 2) // BANKP, NBANKS - 1)
    gnode = bank1 * 2 + (row1 % 2)  # frozen (bank,parity) label per node

    # per-node group-count vectors (node as dst)
    kvec = np.zeros((N, 4), np.int64)
    np.add.at(kvec, (dst, gnode[src]), 1)

    # ---- phase 2: repack within row-classes by vector key
    pos_of = np.zeros(N, np.int64)
    ords = []
    kmax = kvec.max(axis=1)
    for c in range(NCORES):
        lo, hi = c * nsh, min((c + 1) * nsh, N)
        nodes = np.arange(lo, hi)
        nreal = len(nodes)
        # class of each position (same formula as row1)
        posi = np.arange(nreal)
        rowp = c * npad + (posi % P) * T2 + posi // P
        clsp = np.minimum((rowp // 2) // BANKP, NBANKS - 1) * 2 + (rowp % 2)
        order_full = np.zeros(nreal, np.int64)
        for C in range(4):
            nn = nodes[gnode[nodes] == C]
            key = np.lexsort(
                (kvec[nn, 3], kvec[nn, 2], kvec[nn, 1], kvec[nn, 0], kmax[nn])
            )
            nn_sorted = nn[key]
            pp = posi[clsp == C]
            order_full[pp] = nn_sorted
        pos_of[order_full] = posi
        ords.append(order_full)

    table_row = n_owner * npad + (pos_of % P) * T2 + pos_of // P
    pair_of = table_row // 2
    bank_of = np.minimum(pair_of // BANKP, NBANKS - 1)
    # group labels stay consistent: repack preserved (bank,parity)
    g_of = bank_of * 2 + (table_row % 2)

    # ---- per-core edges with (t, p, g, rank)
    percore = []
    Lg = np.zeros((T, 4), np.int64)
    for c in range(NCORES):
        m = np.minimum(dst // nsh, NCORES - 1) == c
        s_c = src[m]
        pos = pos_of[dst[m]]
        g = g_of[s_c]
        t_arr = pos // P
        p_arr = pos % P
        key = (pos * 4 + g)
        order = np.argsort(key, kind="stable")
        s_c, pos, g, t_arr, p_arr = (
            s_c[order], pos[order], g[order], t_arr[order], p_arr[order]
        )
        kk = key[order]
        first = np.concatenate([[True], kk[1:] != kk[:-1]])
        idxs = np.arange(len(kk))
        start = np.maximum.accumulate(np.where(first, idxs, 0))
        rank = idxs - start
        percore.append((s_c, t_arr, p_arr, g, rank))
        cnt = np.zeros((T, P, 4), np.int64)
        np.add.at(cnt, (t_arr, p_arr, g), 1)
        Lg = np.maximum(Lg, cnt.max(axis=1))
    Lg = np.maximum(Lg, 1)

    # ---- chunks: consecutive tiles; per-group L = max over range
    chunks = []
    colbase = 0
    t0 = 0
    while t0 < T:
        nt = 1
        while t0 + nt < T and nt < MAXNT:
            Ls_try = Lg[t0 : t0 + nt + 1].max(axis=0)
            if int(Ls_try.sum()) * (nt + 1) > MPC_COLS:
                break
            nt += 1
        Ls = Lg[t0 : t0 + nt].max(axis=0).astype(np.int64)
        gbs = np.concatenate([[0], np.cumsum(Ls * nt)])
        cols = int(gbs[-1])
        calls = []
        for g in range(4):
            a = int(gbs[g])
            end = int(gbs[g + 1])
            while a < end:
                b = min(a + CALL_COLS, end)
                calls.append((a, b - a, g))
                a = b
        chunks.append(dict(t0=t0, nt=nt, Ls=[int(x) for x in Ls],
                           gbs=[int(x) for x in gbs], cb=colbase, calls=calls))
        colbase += cols
        t0 += nt
    NSLOT = colbase

    # tile -> chunk lookup arrays
    cb_of = np.zeros(T, np.int64)
    nt_of = np.zeros(T, np.int64)
    t0_of = np.zeros(T, np.int64)
    gb_of = np.zeros((T, 4), np.int64)
    for ch in chunks:
        for t in range(ch["t0"], ch["t0"] + ch["nt"]):
            cb_of[t] = ch["cb"]
            nt_of[t] = ch["nt"]
            t0_of[t] = ch["t0"]
            gb_of[t] = ch["gbs"][:4]

    # ---- pad pairs: per (bank, parity) a pair whose q-half row is zero
    pad_pair = np.zeros((NBANKS, 2), np.int64)
    zc, zp = np.meshgrid(np.arange(NCORES), np.arange(P), indexing="ij")
    zrows = (zc * npad + zp * T2 + T).ravel()
    for b in range(NBANKS):
        for q in range(2):
            cand = zrows[
                (zrows % 2 == q)
                & (np.minimum((zrows // 2) // BANKP, NBANKS - 1) == b)
            ]
            assert len(cand) > 0
            pad_pair[b, q] = cand[0] // 2 - b * BANKP

    dWs = (dW.astype(np.float32) * (SIGMA * sqrt_dt)).astype(np.float32)

    in_maps = []
    for c in range(NCORES):
        s_c, t_arr, p_arr, g_arr, rank = percore[c]
        # slot index per edge
        col = (
            cb_of[t_arr]
            + gb_of[t_arr, 0] * 0
            + np.choose(g_arr, gb_of[t_arr].T)
            + rank * nt_of[t_arr]
            + (t_arr - t0_of[t_arr])
        )
        slot = col * P + p_arr
        # init pidx values per column by group pad, then scatter real edges
        vals = np.zeros(NSLOT * P, np.int16)
        for ch in chunks:
            for g in range(4):
                b, q = divmod(g, 2)
                c0 = ch["cb"] + ch["gbs"][g]
                c1 = ch["cb"] + ch["gbs"][g + 1]
                vals[c0 * P : c1 * P] = np.int16(pad_pair[b, q])
        bankv = g_arr // 2
        vals[slot] = (pair_of[s_c] - bankv * BANKP).astype(np.int16)
        pidx = np.zeros((P, NSLOT * P // 16), np.int16)
        sl = np.arange(NSLOT * P)
        for rep in range(8):
            pidx[(sl % 16) + rep * 16, sl // 16] = vals
        del vals

        o = ords[c]
        nreal = len(o)
        ytmp = np.zeros((T * P, HID), np.float32)
        ytmp[:nreal] = x0[o]
        y0 = ytmp.reshape(T, P, HID).transpose(1, 0, 2).reshape(P, WY)
        rv = np.zeros(T * P, np.float32)
        rv[:nreal] = rdi[o] * DT
        rdi_full = (
            np.repeat(rv, HID).reshape(T, P, HID).transpose(1, 0, 2).reshape(P, WY)
        ).astype(ml_dtypes.bfloat16)
        ro = np.zeros(T * P, np.float32)
        ro[:nreal] = rdo[o]
        rdo_col = ro.reshape(T, P).T.copy()
        dtmp = np.zeros((STEPS, T * P, HID), np.float32)
        dtmp[:, :nreal] = dWs[:, o, :]
        dwc = (
            dtmp.reshape(STEPS, T, P, HID)
            .transpose(0, 2, 1, 3)
            .reshape(STEPS, P, WY)
            .astype(ml_dtypes.bfloat16)
        )
        in_maps.append(
            {
                "y0_in": np.ascontiguousarray(y0),
                "dw_in": np.ascontiguousarray(dwc),
                "rdi_in": np.ascontiguousarray(rdi_full),
                "rdo_in": np.ascontiguousarray(rdo_col),
                "pidx_in": np.ascontiguousarray(pidx),
                "wmsg_in": W_msg.astype(np.float32),
                "wout_in": W_out.astype(np.float32),
            }
        )

    meta = dict(
        N=N, HID=HID, OUTF=W_out.shape[1], STEPS=STEPS, DT=DT, T=T, T2=T2,
        npad=npad, TBL=TBL, NPAIR=NPAIR, BANKP=BANKP, NSLOT=NSLOT, WY=WY,
        W2=W2, chunks=chunks, ords=ords, nsh=nsh,
    )
    return in_maps, meta


def _build(meta):
    from concourse.masks import make_identity

    HID, OUTF, STEPS, DT = meta["HID"], meta["OUTF"], meta["STEPS"], meta["DT"]
    T, T2, TBL = meta["T"], meta["T2"], meta["TBL"]
    NSLOT, WY, W2 = meta["NSLOT"], meta["WY"], meta["W2"]
    BANKP = meta["BANKP"]
    chunks = meta["chunks"]
    QT = (T + 3) // 4
    HOUT = QT * OUTF
    E2 = 2 * HID  # 128 bf16 values per pair element

    nc = bacc.Bacc("TRN2", target_bir_lowering=False, debug=False,
                   num_devices=NCORES, num_swdge_queues=NQ)
    y0_in = nc.dram_tensor("y0_in", [P, WY], mybir.dt.float32, kind="ExternalInput")
    dw_in = nc.dram_tensor("dw_in", [STEPS, P, WY], mybir.dt.bfloat16, kind="ExternalInput")
    rdi_in = nc.dram_tensor("rdi_in", [P, WY], mybir.dt.bfloat16, kind="ExternalInput")
    rdo_in = nc.dram_tensor("rdo_in", [P, T], mybir.dt.float32, kind="ExternalInput")
    pidx_in = nc.dram_tensor("pidx_in", [P, NSLOT * 8], mybir.dt.int16, kind="ExternalInput")
    wmsg_in = nc.dram_tensor("wmsg_in", [HID, HID], mybir.dt.float32, kind="ExternalInput")
    wout_in = nc.dram_tensor("wout_in", [HID, OUTF], mybir.dt.float32, kind="ExternalInput")
    out_d = nc.dram_tensor("out_d", [P, T * OUTF], mybir.dt.float32, kind="ExternalOutput")

    with tile.TileContext(nc) as tc:
        with (
            tc.tile_pool(name="state", bufs=1) as st,
            tc.tile_pool(name="work", bufs=3) as wk,
            tc.tile_pool(name="dwp", bufs=2) as dwp,
            tc.tile_pool(name="mpcp", bufs=2) as mpcp,
            tc.tile_pool(name="pxp", bufs=2) as pxp,
            tc.tile_pool(name="osp", bufs=1) as osp,
            tc.tile_pool(name="psum", bufs=2, space="PSUM") as ps,
            tc.tile_pool(name="dram", bufs=1, space="DRAM") as dram,
        ):
            y = st.tile([P, WY], mybir.dt.float32)
            nc.sync.dma_start(y[:], y0_in[:, :])
            rdi_t = st.tile([P, WY], mybir.dt.bfloat16)
            nc.sync.dma_start(rdi_t[:], rdi_in[:, :])
            rdo_t = st.tile([P, T], mybir.dt.float32)
            nc.sync.dma_start(rdo_t[:], rdo_in[:, :])
            wmsg = st.tile([HID, HID], mybir.dt.float32)
            nc.sync.dma_start(wmsg[:], wmsg_in[:, :])
            wout = st.tile([HID, OUTF], mybir.dt.float32)
            nc.sync.dma_start(wout[:], wout_in[:, :])
            ident = st.tile([P, P], mybir.dt.float32)
            make_identity(nc, ident[:])
            m_stage = st.tile([P, W2], mybir.dt.bfloat16)
            nc.vector.memset(m_stage[:], 0.0)
            agg = st.tile([P, WY], mybir.dt.float32)

            table = dram.tile([TBL, HID], mybir.dt.bfloat16)
            bounce = dram.tile([P, W2], mybir.dt.bfloat16)
            tpair = table[:, :].rearrange("(r two) h -> r (two h)", two=2)

            def emit_msgs(t0, nt):
                for t in range(t0, t0 + nt):
                    ytp = ps.tile([HID, P], mybir.dt.float32, space="PSUM", tag="ytp")
                    nc.tensor.transpose(
                        out=ytp[:], in_=y[:, t * HID : (t + 1) * HID], identity=ident[:]
                    )
                    yT = wk.tile([HID, P], mybir.dt.float32, tag="yT")
                    nc.scalar.activation(yT[:], ytp[:], mybir.ActivationFunctionType.Copy)
                    mp = ps.tile([P, HID], mybir.dt.float32, space="PSUM", tag="mp")
                    nc.tensor.matmul(out=mp[:], lhsT=yT[:], rhs=wmsg[:], start=True, stop=True)
                    nc.scalar.activation(
                        m_stage[:, t * HID : (t + 1) * HID],
                        mp[:],
                        mybir.ActivationFunctionType.Copy,
                        scale=rdo_t[:, t : t + 1],
                    )

            def emit_ag():
                nc.sync.dma_start(bounce[:], m_stage[:])
                nc.gpsimd.collective_compute(
                    "AllGather",
                    mybir.AluOpType.bypass,
                    replica_groups=[list(range(NCORES))],
                    ins=[bounce[:]],
                    outs=[table[0:TBL, :]],
                )

            emit_msgs(0, T)
            emit_ag()

            qrot = 0
            for k in range(STEPS):
                for ch in chunks:
                    t0, nt, cb = ch["t0"], ch["nt"], ch["cb"]
                    Ls, gbs = ch["Ls"], ch["gbs"]
                    cols = gbs[4]
                    W64 = nt * HID
                    sl = slice(t0 * HID, (t0 + nt) * HID)
                    dwc = dwp.tile([P, MAXNT * HID], mybir.dt.bfloat16, tag="dw")
                    nc.sync.dma_start(dwc[:, 0:W64], dw_in[k, :, sl])
                    pix = pxp.tile([P, MPC_COLS * 8], mybir.dt.int16, tag="pix")
                    nc.sync.dma_start(
                        pix[:, 0 : cols * 8],
                        pidx_in[:, cb * 8 : (cb + cols) * 8],
                    )
                    mpc = mpcp.tile([P, MPC_COLS * E2], mybir.dt.bfloat16, tag="mpc")
                    for (a, ncol, g) in ch["calls"]:
                        b = g // 2
                        nidx = ncol * P
                        nc.gpsimd.dma_gather(
                            out_ap=mpc[:, a * E2 : (a + ncol) * E2].rearrange(
                                "p (c e) -> p c e", c=ncol
                            ),
                            in_ap=tpair[b * BANKP : (b + 1) * BANKP, :],
                            idxs_ap=pix[:, a * 8 : (a + ncol) * 8],
                            num_idxs=nidx,
                            num_idxs_reg=nidx,
                            elem_size=E2,
                            queue_num=1 + (qrot % (NQ - 1)),
                        )
                        qrot += 1
                    # per-group in-place contiguous tree over k-major rows
                    for g in range(4):
                        gb = gbs[g]
                        cur = Ls[g]
                        while cur > 1:
                            half = cur // 2
                            hi = cur - half
                            nc.vector.tensor_add(
                                mpc[:, gb * E2 : (gb + half * nt) * E2],
                                mpc[:, gb * E2 : (gb + half * nt) * E2],
                                mpc[:, (gb + hi * nt) * E2 : (gb + cur * nt) * E2],
                            )
                            cur = hi
                    # final combine: select parity half, sum 4 groups -> fp32
                    def half_ap(g):
                        q = g % 2
                        gb = gbs[g]
                        return mpc[:, gb * E2 : (gb + nt) * E2].rearrange(
                            "p (c e) -> p c e", c=nt
                        )[:, :, q * HID : (q + 1) * HID]

                    nc.vector.tensor_add(
                        agg[:, sl].rearrange("p (c h) -> p c h", c=nt),
                        half_ap(0), half_ap(1),
                    )
                    for g in (2, 3):
                        nc.vector.tensor_add(
                            agg[:, sl].rearrange("p (c h) -> p c h", c=nt),
                            agg[:, sl].rearrange("p (c h) -> p c h", c=nt),
                            half_ap(g),
                        )
                    # fused update
                    nc.vector.tensor_mul(agg[:, sl], agg[:, sl], rdi_t[:, sl])
                    nc.vector.scalar_tensor_tensor(
                        out=y[:, sl], in0=y[:, sl], scalar=1.0 - DT,
                        in1=agg[:, sl],
                        op0=mybir.AluOpType.mult, op1=mybir.AluOpType.add,
                    )
                    nc.vector.tensor_add(y[:, sl], y[:, sl], dwc[:, 0:W64])
                    if k < STEPS - 1:
                        emit_msgs(t0, nt)
                if k < STEPS - 1:
                    emit_ag()

            for hstart in range(0, T, QT):
                hend = min(hstart + QT, T)
                ostage = osp.tile([P, HOUT], mybir.dt.float32, tag="ostage")
                for t in range(hstart, hend):
                    ytp = ps.tile([HID, P], mybir.dt.float32, space="PSUM", tag="ytp")
                    nc.tensor.transpose(
                        out=ytp[:], in_=y[:, t * HID : (t + 1) * HID], identity=ident[:]
                    )
                    yT = wk.tile([HID, P], mybir.dt.float32, tag="yT")
                    nc.scalar.activation(yT[:], ytp[:], mybir.ActivationFunctionType.Copy)
                    op = ps.tile([P, OUTF], mybir.dt.float32, space="PSUM", tag="op")
                    nc.tensor.matmul(out=op[:], lhsT=yT[:], rhs=wout[:], start=True, stop=True)
                    nc.scalar.activation(
                        ostage[:, (t - hstart) * OUTF : (t - hstart + 1) * OUTF],
                        op[:],
                        mybir.ActivationFunctionType.Copy,
                    )
                nc.sync.dma_start(
                    out_d[:, hstart * OUTF : hend * OUTF],
                    ostage[:, 0 : (hend - hstart) * OUTF],
                )

    nc.compile()
    return nc


def kernel(h, W_in, W_msg, W_out, dW, src, dst):
    h = np.asarray(h)
    W_in = np.asarray(W_in)
    W_msg = np.asarray(W_msg)
    W_out = np.asarray(W_out)
    dW = np.asarray(dW)
    src = np.asarray(src)
    dst = np.asarray(dst)

    in_maps, meta = _preprocess(h, W_in, W_msg, W_out, dW, src, dst)
    nc = _build(meta)

    trace = bool(int(os.environ.get("BASS_KERNEL_TRACE", "0")))
    res = bass_utils.run_bass_kernel_spmd(
        nc, in_maps, core_ids=list(range(NCORES)), trace=trace
    )
    if trace and res.exec_time_ns:
        print(f"HW exec time: {res.exec_time_ns} ns")

    N, OUTF, T = meta["N"], meta["OUTF"], meta["T"]
    out = np.zeros((N, OUTF), np.float32)
    for c in range(NCORES):
        o = meta["ords"][c]
        dev = res.results[c]["out_d"]
        dev = dev.reshape(P, T, OUTF).transpose(1, 0, 2).reshape(T * P, OUTF)
        out[o] = dev[: len(o)]
    return out


# revision 5
# speedup vs baseline: 1.6581x; 1.1694x over previous
"""Trainium2 Bass kernel for BronxModel (GNN message passing SDE).

Strategy (8 NeuronCores, SPMD, dst-sharded nodes):
  - Per SDE step: per-tile PE matmul y@W_msg scaled by rsqrt(deg_out) ->
    bf16 message table shard; AllGather -> DRAM table, viewed as 256B
    pair rows [NPAIR, 128].
  - All per-edge fetches go through dma_gather (packed SWDGE descriptors,
    1024 idxs/call — the HW cap): idx i -> out (partition i%128, column
    i//128) of a chunk staging tile. int16 idxs force 2 banks x row
    parity = 4 edge groups; a slot grid per group with per-tile uniform
    depth L_g(t). Nodes are re-ordered within (bank,parity) row-classes
    by their group-count vectors so tiles hold vector-similar nodes,
    minimizing the max-over-partitions padding.
  - Segment sum: per (chunk, group), an in-place contiguous bf16
    tensor_tensor tree over k-major full 256B elements; the row-parity
    half is selected only at the final fp32 combine. Fused y update.
  - h @ W_in on host; final y @ W_out on device.
"""
import os
import sys

sys.path.insert(0, "/opt/trn_rl_repo")

import numpy as np
import ml_dtypes

import concourse.bass as bass
import concourse.bacc as bacc
import concourse.tile as tile
import concourse.mybir as mybir
from concourse import bass_utils

NCORES = 8
P = 128
NBANKS = 2
CALL_COLS = 8       # grid columns per dma_gather call (8*128 = 1024 idx cap)
MAXNT = 4           # tiles per chunk
MPC_COLS = 104      # max grid columns per chunk (mpc: cols*256B/partition)
NQ = 4              # SWDGE queues; gathers rotate on 1..3


def _preprocess(h, W_in, W_msg, W_out, dW, src, dst):
    N = h.shape[0]
    HID = W_msg.shape[0]
    STEPS = dW.shape[0]
    DT = 1.0 / STEPS
    SIGMA = 0.01
    sqrt_dt = np.sqrt(DT).astype(np.float32)

    nsh = (N + NCORES - 1) // NCORES
    T = (nsh + P - 1) // P
    T2 = T + 1
    npad = T2 * P
    TBL = NCORES * npad
    NPAIR = TBL // 2
    BANKP = (NPAIR + NBANKS - 1) // NBANKS
    WY = T * HID
    W2 = T2 * HID

    ones = np.ones(src.shape[0], np.float32)
    deg_out = np.zeros(N, np.float32)
    np.add.at(deg_out, src, ones)
    deg_in = np.zeros(N, np.float32)
    np.add.at(deg_in, dst, ones)
    rdo = 1.0 / np.sqrt(np.maximum(deg_out, 1.0))
    rdi = 1.0 / np.sqrt(np.maximum(deg_in, 1.0))

    x0 = h.astype(np.float32) @ W_in.astype(np.float32)

    n_owner = np.minimum(np.arange(N) // nsh, NCORES - 1)

    # ---- phase 1: provisional order by in-degree -> frozen group labels
    pos1 = np.zeros(N, np.int64)
    for c in range(NCORES):
        lo, hi = c * nsh, min((c + 1) * nsh, N)
        nodes = np.arange(lo, hi)
        o = nodes[np.argsort(deg_in[nodes], kind="stable")]
        pos1[o] = np.arange(len(o))
    row1 = n_owner * npad + (pos1 % P) * T2 + pos1 // P
    bank1 = np.minimum((row1 // 2) // BANKP, NBANKS - 1)
    gnode = bank1 * 2 + (row1 % 2)  # frozen (bank,parity) label per node

    # per-node group-count vectors (node as dst)
    kvec = np.zeros((N, 4), np.int64)
    np.add.at(kvec, (dst, gnode[src]), 1)

    # ---- phase 2: repack within row-classes by vector key
    pos_of = np.zeros(N, np.int64)
    ords = []
    kmax = kvec.max(axis=1)
    for c in range(NCORES):
        lo, hi = c * nsh, min((c + 1) * nsh, N)
        nodes = np.arange(lo, hi)
        nreal = len(nodes)
        # class of each position (same formula as row1)
        posi = np.arange(nreal)
        rowp = c * npad + (posi % P) * T2 + posi // P
        clsp = np.minimum((rowp // 2) // BANKP, NBANKS - 1) * 2 + (rowp % 2)
        order_full = np.zeros(nreal, np.int64)
        for C in range(4):
            nn = nodes[gnode[nodes] == C]
            key = np.lexsort(
                (kvec[nn, 3], kvec[nn, 2], kvec[nn, 1], kvec[nn, 0], kmax[nn])
            )
            nn_sorted = nn[key]
            pp = posi[clsp == C]
            order_full[pp] = nn_sorted
        pos_of[order_full] = posi
        ords.append(order_full)

    table_row = n_owner * npad + (pos_of % P) * T2 + pos_of // P
    pair_of = table_row // 2
    bank_of = np.minimum(pair_of // BANKP, NBANKS - 1)
    # group labels stay consistent: repack preserved (bank,parity)
    g_of = bank_of * 2 + (table_row % 2)

    # ---- per-core edges with (t, p, g, rank)
    percore = []
    Lg = np.zeros((T, 4), np.int64)
    for c in range(NCORES):
        m = np.minimum(dst // nsh, NCORES - 1) == c
        s_c = src[m]
        pos = pos_of[dst[m]]
        g = g_of[s_c]
        t_arr = pos // P
        p_arr = pos % P
        key = (pos * 4 + g)
        order = np.argsort(key, kind="stable")
        s_c, pos, g, t_arr, p_arr = (
            s_c[order], pos[order], g[order], t_arr[order], p_arr[order]
        )
        kk = key[order]
        first = np.concatenate([[True], kk[1:] != kk[:-1]])
        idxs = np.arange(len(kk))
        start = np.maximum.accumulate(np.where(first, idxs, 0))
        rank = idxs - start
        percore.append((s_c, t_arr, p_arr, g, rank))
        cnt = np.zeros((T, P, 4), np.int64)
        np.add.at(cnt, (t_arr, p_arr, g), 1)
        Lg = np.maximum(Lg, cnt.max(axis=1))
    Lg = np.maximum(Lg, 1)

    # ---- chunks: consecutive tiles; per-group L = max over range
    chunks = []
    colbase = 0
    t0 = 0
    while t0 < T:
        nt = 1
        while t0 + nt < T and nt < MAXNT:
            Ls_try = Lg[t0 : t0 + nt + 1].max(axis=0)
            if int(Ls_try.sum()) * (nt + 1) > MPC_COLS:
                break
            nt += 1
        Ls = Lg[t0 : t0 + nt].max(axis=0).astype(np.int64)
        gbs = np.concatenate([[0], np.cumsum(Ls * nt)])
        cols = int(gbs[-1])
        calls = []
        for g in range(4):
            a = int(gbs[g])
            end = int(gbs[g + 1])
            while a < end:
                b = min(a + CALL_COLS, end)
                calls.append((a, b - a, g))
                a = b
        chunks.append(dict(t0=t0, nt=nt, Ls=[int(x) for x in Ls],
                           gbs=[int(x) for x in gbs], cb=colbase, calls=calls))
        colbase += cols
        t0 += nt
    NSLOT = colbase

    # tile -> chunk lookup arrays
    cb_of = np.zeros(T, np.int64)
    nt_of = np.zeros(T, np.int64)
    t0_of = np.zeros(T, np.int64)
    gb_of = np.zeros((T, 4), np.int64)
    for ch in chunks:
        for t in range(ch["t0"], ch["t0"] + ch["nt"]):
            cb_of[t] = ch["cb"]
            nt_of[t] = ch["nt"]
            t0_of[t] = ch["t0"]
            gb_of[t] = ch["gbs"][:4]

    # ---- pad pairs: per (bank, parity) a pair whose q-half row is zero
    pad_pair = np.zeros((NBANKS, 2), np.int64)
    zc, zp = np.meshgrid(np.arange(NCORES), np.arange(P), indexing="ij")
    zrows = (zc * npad + zp * T2 + T).ravel()
    for b in range(NBANKS):
        for q in range(2):
            cand = zrows[
                (zrows % 2 == q)
                & (np.minimum((zrows // 2) // BANKP, NBANKS - 1) == b)
            ]
            assert len(cand) > 0
            pad_pair[b, q] = cand[0] // 2 - b * BANKP

    dWs = (dW.astype(np.float32) * (SIGMA * sqrt_dt)).astype(np.float32)

    in_maps = []
    for c in range(NCORES):
        s_c, t_arr, p_arr, g_arr, rank = percore[c]
        # slot index per edge
        col = (
            cb_of[t_arr]
            + gb_of[t_arr, 0] * 0
            + np.choose(g_arr, gb_of[t_arr].T)
            + rank * nt_of[t_arr]
            + (t_arr - t0_of[t_arr])
        )
        slot = col * P + p_arr
        # init pidx values per column by group pad, then scatter real edges
        vals = np.zeros(NSLOT * P, np.int16)
        for ch in chunks:
            for g in range(4):
                b, q = divmod(g, 2)
                c0 = ch["cb"] + ch["gbs"][g]
                c1 = ch["cb"] + ch["gbs"][g + 1]
                vals[c0 * P : c1 * P] = np.int16(pad_pair[b, q])
        bankv = g_arr // 2
        vals[slot] = (pair_of[s_c] - bankv * BANKP).astype(np.int16)
        pidx = np.zeros((P, NSLOT * P // 16), np.int16)
        sl = np.arange(NSLOT * P)
        for rep in range(8):
            pidx[(sl % 16) + rep * 16, sl // 16] = vals
        del vals

        o = ords[c]
        nreal = len(o)
        ytmp = np.zeros((T * P, HID), np.float32)
        ytmp[:nreal] = x0[o]
        y0 = ytmp.reshape(T, P, HID).transpose(1, 0, 2).reshape(P, WY)
        rv = np.zeros(T * P, np.float32)
        rv[:nreal] = rdi[o] * DT
        rdi_full = (
            np.repeat(rv, HID).reshape(T, P, HID).transpose(1, 0, 2).reshape(P, WY)
        ).astype(ml_dtypes.bfloat16)
        ro = np.zeros(T * P, np.float32)
        ro[:nreal] = rdo[o]
        rdo_col = ro.reshape(T, P).T.copy()
        dtmp = np.zeros((STEPS, T * P, HID), np.float32)
        dtmp[:, :nreal] = dWs[:, o, :]
        dwc = (
            dtmp.reshape(STEPS, T, P, HID)
            .transpose(0, 2, 1, 3)
            .reshape(STEPS, P, WY)
            .astype(ml_dtypes.bfloat16)
        )
        in_maps.append(
            {
                "y0_in": np.ascontiguousarray(y0),
                "dw_in": np.ascontiguousarray(dwc),
                "rdi_in": np.ascontiguousarray(rdi_full),
                "rdo_in": np.ascontiguousarray(rdo_col),
                "pidx_in": np.ascontiguousarray(pidx),
                "wmsg_in": W_msg.astype(np.float32),
                "wout_in": W_out.astype(np.float32),
            }
        )

    meta = dict(
        N=N, HID=HID, OUTF=W_out.shape[1], STEPS=STEPS, DT=DT, T=T, T2=T2,
        npad=npad, TBL=TBL, NPAIR=NPAIR, BANKP=BANKP, NSLOT=NSLOT, WY=WY,
        W2=W2, chunks=chunks, ords=ords, nsh=nsh,
    )
    return in_maps, meta


def _build(meta):
    from concourse.masks import make_identity

    HID, OUTF, STEPS, DT = meta["HID"], meta["OUTF"], meta["STEPS"], meta["DT"]
    T, T2, TBL = meta["T"], meta["T2"], meta["TBL"]
    NSLOT, WY, W2 = meta["NSLOT"], meta["WY"], meta["W2"]
    BANKP = meta["BANKP"]
    chunks = meta["chunks"]
    QT = (T + 3) // 4
    HOUT = QT * OUTF
    E2 = 2 * HID  # 128 bf16 values per pair element

    nc = bacc.Bacc("TRN2", target_bir_lowering=False, debug=False,
                   num_devices=NCORES, num_swdge_queues=NQ)
    y0_in = nc.dram_tensor("y0_in", [P, WY], mybir.dt.float32, kind="ExternalInput")
    dw_in = nc.dram_tensor("dw_in", [STEPS, P, WY], mybir.dt.bfloat16, kind="ExternalInput")
    rdi_in = nc.dram_tensor("rdi_in", [P, WY], mybir.dt.bfloat16, kind="ExternalInput")
    rdo_in = nc.dram_tensor("rdo_in", [P, T], mybir.dt.float32, kind="ExternalInput")
    pidx_in = nc.dram_tensor("pidx_in", [P, NSLOT * 8], mybir.dt.int16, kind="ExternalInput")
    wmsg_in = nc.dram_tensor("wmsg_in", [HID, HID], mybir.dt.float32, kind="ExternalInput")
    wout_in = nc.dram_tensor("wout_in", [HID, OUTF], mybir.dt.float32, kind="ExternalInput")
    out_d = nc.dram_tensor("out_d", [P, T * OUTF], mybir.dt.float32, kind="ExternalOutput")

    with tile.TileContext(nc) as tc:
        with (
            tc.tile_pool(name="state", bufs=1) as st,
            tc.tile_pool(name="work", bufs=3) as wk,
            tc.tile_pool(name="dwp", bufs=2) as dwp,
            tc.tile_pool(name="mpcp", bufs=3) as mpcp,
            tc.tile_pool(name="pxp", bufs=3) as pxp,
            tc.tile_pool(name="osp", bufs=1) as osp,
            tc.tile_pool(name="psum", bufs=2, space="PSUM") as ps,
            tc.tile_pool(name="dram", bufs=1, space="DRAM") as dram,
        ):
            y = st.tile([P, WY], mybir.dt.float32)
            nc.sync.dma_start(y[:], y0_in[:, :])
            rdi_t = st.tile([P, WY], mybir.dt.bfloat16)
            nc.sync.dma_start(rdi_t[:], rdi_in[:, :])
            rdo_t = st.tile([P, T], mybir.dt.float32)
            nc.sync.dma_start(rdo_t[:], rdo_in[:, :])
            wmsg = st.tile([HID, HID], mybir.dt.float32)
            nc.sync.dma_start(wmsg[:], wmsg_in[:, :])
            wout = st.tile([HID, OUTF], mybir.dt.float32)
            nc.sync.dma_start(wout[:], wout_in[:, :])
            ident = st.tile([P, P], mybir.dt.float32)
            make_identity(nc, ident[:])
            m_stage = st.tile([P, W2], mybir.dt.bfloat16)
            nc.vector.memset(m_stage[:], 0.0)
            agg = st.tile([P, WY], mybir.dt.float32)

            table = dram.tile([TBL, HID], mybir.dt.bfloat16)
            bounce = dram.tile([P, W2], mybir.dt.bfloat16)
            tpair = table[:, :].rearrange("(r two) h -> r (two h)", two=2)

            def emit_msgs(t0, nt):
                for t in range(t0, t0 + nt):
                    ytp = ps.tile([HID, P], mybir.dt.float32, space="PSUM", tag="ytp")
                    nc.tensor.transpose(
                        out=ytp[:], in_=y[:, t * HID : (t + 1) * HID], identity=ident[:]
                    )
                    yT = wk.tile([HID, P], mybir.dt.float32, tag="yT")
                    nc.scalar.activation(yT[:], ytp[:], mybir.ActivationFunctionType.Copy)
                    mp = ps.tile([P, HID], mybir.dt.float32, space="PSUM", tag="mp")
                    nc.tensor.matmul(out=mp[:], lhsT=yT[:], rhs=wmsg[:], start=True, stop=True)
                    nc.scalar.activation(
                        m_stage[:, t * HID : (t + 1) * HID],
                        mp[:],
                        mybir.ActivationFunctionType.Copy,
                        scale=rdo_t[:, t : t + 1],
                    )

            def emit_ag():
                nc.sync.dma_start(bounce[:], m_stage[:])
                nc.gpsimd.collective_compute(
                    "AllGather",
                    mybir.AluOpType.bypass,
                    replica_groups=[list(range(NCORES))],
                    ins=[bounce[:]],
                    outs=[table[0:TBL, :]],
                )

            emit_msgs(0, T)
            emit_ag()

            qrot = 0
            for k in range(STEPS):
                for ch in chunks:
                    t0, nt, cb = ch["t0"], ch["nt"], ch["cb"]
                    Ls, gbs = ch["Ls"], ch["gbs"]
                    cols = gbs[4]
                    W64 = nt * HID
                    sl = slice(t0 * HID, (t0 + nt) * HID)
                    dwc = dwp.tile([P, MAXNT * HID], mybir.dt.bfloat16, tag="dw")
                    nc.sync.dma_start(dwc[:, 0:W64], dw_in[k, :, sl])
                    pix = pxp.tile([P, MPC_COLS * 8], mybir.dt.int16, tag="pix")
                    nc.sync.dma_start(
                        pix[:, 0 : cols * 8],
                        pidx_in[:, cb * 8 : (cb + cols) * 8],
                    )
                    mpc = mpcp.tile([P, MPC_COLS * E2], mybir.dt.bfloat16, tag="mpc")
                    for (a, ncol, g) in ch["calls"]:
                        b = g // 2
                        nidx = ncol * P
                        nc.gpsimd.dma_gather(
                            out_ap=mpc[:, a * E2 : (a + ncol) * E2].rearrange(
                                "p (c e) -> p c e", c=ncol
                            ),
                            in_ap=tpair[b * BANKP : (b + 1) * BANKP, :],
                            idxs_ap=pix[:, a * 8 : (a + ncol) * 8],
                            num_idxs=nidx,
                            num_idxs_reg=nidx,
                            elem_size=E2,
                            queue_num=qrot % NQ,
                        )
                        qrot += 1
                    # per-group in-place contiguous tree over k-major rows
                    for g in range(4):
                        gb = gbs[g]
                        cur = Ls[g]
                        while cur > 1:
                            half = cur // 2
                            hi = cur - half
                            nc.vector.tensor_add(
                                mpc[:, gb * E2 : (gb + half * nt) * E2],
                                mpc[:, gb * E2 : (gb + half * nt) * E2],
                                mpc[:, (gb + hi * nt) * E2 : (gb + cur * nt) * E2],
                            )
                            cur = hi
                    # final combine: select parity half, sum 4 groups -> fp32
                    def half_ap(g):
                        q = g % 2
                        gb = gbs[g]
                        return mpc[:, gb * E2 : (gb + nt) * E2].rearrange(
                            "p (c e) -> p c e", c=nt
                        )[:, :, q * HID : (q + 1) * HID]

                    nc.vector.tensor_add(
                        agg[:, sl].rearrange("p (c h) -> p c h", c=nt),
                        half_ap(0), half_ap(1),
                    )
                    for g in (2, 3):
                        nc.vector.tensor_add(
                            agg[:, sl].rearrange("p (c h) -> p c h", c=nt),
                            agg[:, sl].rearrange("p (c h) -> p c h", c=nt),
                            half_ap(g),
                        )
                    # fused update
                    nc.vector.tensor_mul(agg[:, sl], agg[:, sl], rdi_t[:, sl])
                    nc.vector.scalar_tensor_tensor(
                        out=y[:, sl], in0=y[:, sl], scalar=1.0 - DT,
                        in1=agg[:, sl],
                        op0=mybir.AluOpType.mult, op1=mybir.AluOpType.add,
                    )
                    nc.vector.tensor_add(y[:, sl], y[:, sl], dwc[:, 0:W64])
                    if k < STEPS - 1:
                        emit_msgs(t0, nt)
                if k < STEPS - 1:
                    emit_ag()

            for hstart in range(0, T, QT):
                hend = min(hstart + QT, T)
                ostage = osp.tile([P, HOUT], mybir.dt.float32, tag="ostage")
                for t in range(hstart, hend):
                    ytp = ps.tile([HID, P], mybir.dt.float32, space="PSUM", tag="ytp")
                    nc.tensor.transpose(
                        out=ytp[:], in_=y[:, t * HID : (t + 1) * HID], identity=ident[:]
                    )
                    yT = wk.tile([HID, P], mybir.dt.float32, tag="yT")
                    nc.scalar.activation(yT[:], ytp[:], mybir.ActivationFunctionType.Copy)
                    op = ps.tile([P, OUTF], mybir.dt.float32, space="PSUM", tag="op")
                    nc.tensor.matmul(out=op[:], lhsT=yT[:], rhs=wout[:], start=True, stop=True)
                    nc.scalar.activation(
                        ostage[:, (t - hstart) * OUTF : (t - hstart + 1) * OUTF],
                        op[:],
                        mybir.ActivationFunctionType.Copy,
                    )
                nc.sync.dma_start(
                    out_d[:, hstart * OUTF : hend * OUTF],
                    ostage[:, 0 : (hend - hstart) * OUTF],
                )

    nc.compile()
    return nc


def kernel(h, W_in, W_msg, W_out, dW, src, dst):
    h = np.asarray(h)
    W_in = np.asarray(W_in)
    W_msg = np.asarray(W_msg)
    W_out = np.asarray(W_out)
    dW = np.asarray(dW)
    src = np.asarray(src)
    dst = np.asarray(dst)

    in_maps, meta = _preprocess(h, W_in, W_msg, W_out, dW, src, dst)
    nc = _build(meta)

    trace = bool(int(os.environ.get("BASS_KERNEL_TRACE", "0")))
    res = bass_utils.run_bass_kernel_spmd(
        nc, in_maps, core_ids=list(range(NCORES)), trace=trace
    )
    if trace and res.exec_time_ns:
        print(f"HW exec time: {res.exec_time_ns} ns")

    N, OUTF, T = meta["N"], meta["OUTF"], meta["T"]
    out = np.zeros((N, OUTF), np.float32)
    for c in range(NCORES):
        o = meta["ords"][c]
        dev = res.results[c]["out_d"]
        dev = dev.reshape(P, T, OUTF).transpose(1, 0, 2).reshape(T * P, OUTF)
        out[o] = dev[: len(o)]
    return out
